# revision 26
# baseline (speedup 1.0000x reference)
"""Leaky-integrator linear recurrence kernel for Trainium2.

u_t = TAU * u_{t-1} + x_t along the last (time) axis of x[32, 1024, 2048] f32.

Strategy: data-parallel across 8 NeuronCores (4 batches each), 16-bit HBM
traffic (the 2e-2 tolerance dwarfs bf16 quantization), and a HYBRID compute
split that balances all engines below the DMA stream time:

* 2560 rows/core go through the Tensor engine as a *banded matmul* in a
  host-transposed layout xt[time, rows]: since TAU^129 < 2e-6, u_t is (to
  float precision) a windowed sum over the last 256 steps, computed per
  128-step block as two accumulating 128x128-stationary matmuls
  (cross-block band A + triangular band B; block 0 skips A). PSUM f32 ->
  SBUF bf16 downcasts for this path run on the Scalar engine (~53 us).
* 1536 rows/core go through the Vector engine's hardware scan
  (TensorTensorScanArith, fp32 internal state) in the natural x[row, time]
  layout, 12 tiles of [128, 2048] at ~4.3 us each (~52 us).

With PE at ~55 us, Vector ~53 us, Scalar ~53 us, the kernel is bound by
the DMA stream itself (~33.6 MB/core over 16 engines at line rate).

Engine assignment: Sync issues input DMAs, Scalar issues output DMAs (two
HWDGE rings, so input prefetch never head-of-line blocks behind output
drain). All DMAs keep full 128-partition alignment — partial-partition
APs defeat balance_dma_aps and serialize a transfer onto one DMA engine.

_dedup_ldweights(): tile_legalize splits each matmul into InstLdweights +
a non-self-loading InstMatmult; consecutive Ldweights with identical
weights APs are redundant (Matmult does not clobber the PE array), so all
but the first are dropped (~100 ns of PE time each).

The walrus build in this container allows at most ONE embedded sync-wait
per engine instruction (two on EventSemaphore); Tile's wait assignment can
attach several. _split_excess_waits() hoists the extras onto standalone
EventSemaphore instructions inserted immediately before, on the same
engine — conservative but correct, since every awaited semaphore's
producer precedes the waiter in the scheduled program order.
"""

import numpy as np
import ml_dtypes

import concourse.bass as bass
import concourse.mybir as mybir
from concourse.bass_utils import run_bass_kernel_spmd
from concourse.tile import TileContext

TAU = 0.9
B, F, T = 32, 1024, 2048
N_CORES = 8
B_PER_CORE = B // N_CORES          # 4
ROWS = B_PER_CORE * F              # 4096 independent recurrences per core
P = 128
N_BLK = T // P                     # 16 time-blocks (slabs)
CHUNK = 512                        # PSUM bank width (f32)

MM_ROWS = 2048                     # rows on the TensorE matmul path
SC_ROWS = ROWS - MM_ROWS           # 2048 rows on the VectorE scan path
N_CHUNK = MM_ROWS // CHUNK         # 4
R_PER_P = 4                        # scan rows packed per partition line
N_SSUP = SC_ROWS // (P * R_PER_P)  # 3 scan super-tiles [128, 4*T]
N_PAIR = N_BLK // 2                # 8 slab pairs on the matmul path

NP_DT = ml_dtypes.bfloat16
MYBIR_DT = mybir.dt.bfloat16
SC_NP_DT = np.float16              # scan path: fp16(0.9)=0.89990 — bf16's
SC_MYBIR_DT = mybir.dt.float16     # 0.8984 compounds over the recurrence

_nc_cache = None
_coef_cache = None
last_results = None  # BassKernelResults from the most recent run (for test.py)


def _split_excess_waits(nc: bass.Bass) -> None:
    for fn in nc.m.functions:
        for blk in fn.blocks:
            out = []
            changed = False
            for inst in blk.instructions:
                si = inst.sync_info
                waits = list(si.on_wait) if si is not None else []
                cap = 2 if inst.opcode == "EventSemaphore" else 1
                if len(waits) <= cap:
                    out.append(inst)
                    continue
                changed = True
                # On DMAs keep a queue-ordering (DMAHW*) wait embedded so
                # queue-level throttling stays at the queue; otherwise keep
                # the last wait.
                keep_idx = len(waits) - 1
                if inst.opcode == "DMACopy":
                    for k, w in enumerate(waits):
                        if (w.ant_name or "").startswith("DMA"):
                            keep_idx = k
                            break
                rest = [w for j, w in enumerate(waits) if j != keep_idx]
                for j in range(0, len(rest), 2):
                    out.append(
                        mybir.InstEventSemaphore(
                            name=f"{inst.name}-xw{j}",
                            opcode="EventSemaphore",
                            engine=inst.engine,
                            debug=inst.debug,
                            sync_info=mybir.SyncInfo(
                                on_wait=rest[j : j + 2], on_update=[]
                            ),
                        )
                    )
                inst.sync_info = mybir.SyncInfo(
                    on_wait=[waits[keep_idx]], on_update=list(si.on_update)
                )
                out.append(inst)
            if changed:
                blk.instructions = out


def _dedup_ldweights(nc: bass.Bass) -> None:
    """Drop PE weight reloads that reload the already-loaded stationary."""
    for fn in nc.m.functions:
        for blk in fn.blocks:
            out = []
            changed = False
            last_sig = None
            for inst in blk.instructions:
                if inst.opcode == "Matmult":
                    out.append(inst)
                    continue
                if inst.opcode != "Ldweights":
                    if inst.engine == mybir.EngineType.PE and inst.opcode not in (
                        "EventSemaphore",
                    ):
                        last_sig = None
                    out.append(inst)
                    continue
                a = inst.ins[0]
                sig = (a.memref, a.offset, str(a.ap), str(a.dtype))
                if sig != last_sig:
                    last_sig = sig
                    out.append(inst)
                    continue
                changed = True
                si = inst.sync_info
                waits = list(si.on_wait) if si is not None else []
                upds = list(si.on_update) if si is not None else []
                if waits or upds:
                    for j in range(0, max(len(waits), 1), 2):
                        out.append(
                            mybir.InstEventSemaphore(
                                name=f"{inst.name}-lw{j}",
                                opcode="EventSemaphore",
                                engine=inst.engine,
                                debug=inst.debug,
                                sync_info=mybir.SyncInfo(
                                    on_wait=waits[j : j + 2],
                                    on_update=upds if j == 0 else [],
                                ),
                            )
                        )
            if changed:
                blk.instructions = out


def _coef() -> np.ndarray:
    # [P, 2P] = [A | B] packed side by side (one SBUF tile, one DMA):
    #   A[k, m] = TAU^(m+128-k)                (cross-block band)
    #   B[k, m] = TAU^(m-k) for k <= m else 0  (triangular band)
    k = np.arange(2 * P)[:, None]
    m = np.arange(P)[None, :]
    e = m + P - k
    c = np.where(e >= 0, TAU ** np.maximum(e, 0).astype(np.float64), 0.0)
    return np.ascontiguousarray(np.hstack([c[:P], c[P:]]).astype(NP_DT))


def _build() -> bass.Bass:
    nc = bass.Bass()
    # Matmul-path tensors are host-interleaved so each [128, 2*MM_ROWS]
    # slab-pair tile has one 10 KiB contiguous HBM run per partition line:
    # xp row q*128+p holds time-rows 256q+p and 256q+128+p side by side.
    # (Un-paired [time, MM_ROWS] layout gives 5 KiB lines — the smaller
    # descriptors cost ~10% of DMA line rate.)
    xp = nc.dram_tensor("xp", [N_PAIR * P, 2 * MM_ROWS], MYBIR_DT, kind="ExternalInput")
    xs = nc.dram_tensor("xs", [SC_ROWS, T], SC_MYBIR_DT, kind="ExternalInput")
    coef = nc.dram_tensor("coef", [P, 2 * P], MYBIR_DT, kind="ExternalInput")
    yp = nc.dram_tensor("yp", [N_PAIR * P, 2 * MM_ROWS], MYBIR_DT, kind="ExternalOutput")
    ys = nc.dram_tensor("ys", [SC_ROWS, T], SC_MYBIR_DT, kind="ExternalOutput")

    xp_r = xp.rearrange("(q p) w -> q p w", p=P)   # 8 pairs [128, 2*MM_ROWS]
    yp_r = yp.rearrange("(q p) w -> q p w", p=P)
    # Scan path: 4 consecutive rows per partition line (16 KiB runs).
    xs_r = xs.rearrange("(i p s) t -> i p (s t)", p=P, s=R_PER_P)
    ys_r = ys.rearrange("(i p s) t -> i p (s t)", p=P, s=R_PER_P)

    with TileContext(nc) as tc:
        with (
            tc.tile_pool(name="const", bufs=1) as cpool,
            tc.tile_pool(name="in", bufs=5) as ipool,
            tc.tile_pool(name="out", bufs=3) as opool,
            tc.tile_pool(name="sin", bufs=2) as sipool,
            tc.tile_pool(name="sout", bufs=2) as sopool,
            tc.tile_pool(name="psum", bufs=8, space="PSUM") as ppool,
        ):
            cf = cpool.tile([P, 2 * P], MYBIR_DT)
            nc.sync.dma_start(out=cf[:], in_=coef[:])
            cA = cf[:, 0:P]
            cB = cf[:, P : 2 * P]
            tau = cpool.tile([P, T], SC_MYBIR_DT)
            nc.vector.memset(tau[:], TAU)

            def scan_supertile(k):
                sin = sipool.tile([P, R_PER_P * T], SC_MYBIR_DT)
                nc.sync.dma_start(out=sin[:], in_=xs_r[k])
                sout = sopool.tile([P, R_PER_P * T], SC_MYBIR_DT)
                for r in range(R_PER_P):
                    nc.vector.tensor_tensor_scan(
                        sout[:, r * T : (r + 1) * T],
                        tau[:],
                        sin[:, r * T : (r + 1) * T],
                        0.0,
                        mybir.AluOpType.mult,
                        mybir.AluOpType.add,
                    )
                nc.scalar.dma_start(out=ys_r[k], in_=sout[:])

            def slab(i):
                # view of time-block i inside its pair tile
                return pairs[i // 2][:, (i % 2) * MM_ROWS : (i % 2 + 1) * MM_ROWS]

            pairs = []
            sk = 0
            for q in range(N_PAIR):
                sp = ipool.tile([P, 2 * MM_ROWS], MYBIR_DT)
                nc.sync.dma_start(out=sp[:], in_=xp_r[q])
                pairs.append(sp)

                upair = opool.tile([P, 2 * MM_ROWS], MYBIR_DT)
                for j in range(2):
                    i = 2 * q + j
                    order = list(range(N_CHUNK))
                    if i % 2:
                        order.reverse()
                    pts = {}
                    for c in order:
                        pt = ppool.tile([P, CHUNK], mybir.dt.float32)
                        pts[c] = pt
                        if i > 0:
                            nc.tensor.matmul(
                                pt[:], lhsT=cA[:],
                                rhs=slab(i - 1)[:, c * CHUNK : (c + 1) * CHUNK],
                                start=True, stop=False,
                            )
                    for c in order:
                        osl = slice(j * MM_ROWS + c * CHUNK, j * MM_ROWS + (c + 1) * CHUNK)
                        nc.tensor.matmul(
                            pts[c][:], lhsT=cB[:],
                            rhs=slab(i)[:, c * CHUNK : (c + 1) * CHUNK],
                            start=(i == 0), stop=True,
                        )
                        nc.scalar.copy(upair[:, osl], pts[c][:])
                        if q == N_PAIR - 1 and j == 1:
                            # stream the final block's output per chunk so
                            # its writes are ready as the read stream ends
                            nc.scalar.dma_start(out=yp_r[q][:, osl], in_=upair[:, osl])
                    if q == N_PAIR - 1 and j == 0:
                        nc.scalar.dma_start(
                            out=yp_r[q][:, 0:MM_ROWS], in_=upair[:, 0:MM_ROWS]
                        )
                if q != N_PAIR - 1:
                    nc.scalar.dma_start(out=yp_r[q], in_=upair[:])
                if q >= 1:
                    pairs[q - 1] = None

                # interleave the 3 scan super-tiles across the 8 pairs
                while sk * N_PAIR < (q + 1) * N_SSUP:
                    scan_supertile(sk)
                    sk += 1

    _dedup_ldweights(nc)
    _split_excess_waits(nc)
    return nc


def kernel(x: np.ndarray, **_unused) -> np.ndarray:
    global _nc_cache, _coef_cache, last_results
    if _nc_cache is None:
        _nc_cache = _build()
        _coef_cache = _coef()
    nc = _nc_cache

    x = np.asarray(x)
    assert x.shape == (B, F, T), x.shape
    xr = x.reshape(N_CORES, ROWS, T)
    in_maps = []
    for c in range(N_CORES):
        # matmul path: transpose to [T, MM_ROWS] then interleave time-rows
        # t and t+128 of each 256-step pair side by side (10 KiB HBM lines)
        xt = xr[c, 0:MM_ROWS].astype(NP_DT).T          # [T, MM_ROWS]
        xp = np.ascontiguousarray(
            xt.reshape(N_PAIR, 2, P, MM_ROWS)
            .transpose(0, 2, 1, 3)
            .reshape(N_PAIR * P, 2 * MM_ROWS)
        )
        xs = np.ascontiguousarray(xr[c, MM_ROWS:], dtype=SC_NP_DT)
        in_maps.append({"xp": xp, "xs": xs, "coef": _coef_cache})
    last_results = run_bass_kernel_spmd(
        nc, in_maps, core_ids=list(range(N_CORES))
    )
    outs = []
    for r in last_results.results:
        yt = (
            r["yp"]
            .reshape(N_PAIR, P, 2, MM_ROWS)
            .transpose(0, 2, 1, 3)
            .reshape(T, MM_ROWS)
        )
        u = np.concatenate(
            [yt.T.astype(np.float32), r["ys"].astype(np.float32)], axis=0
        )
        outs.append(u.reshape(B_PER_CORE, F, T))
    return np.concatenate(outs, axis=0)


# revision 27
# speedup vs baseline: 1.2145x; 1.2145x over previous
"""Leaky-integrator linear recurrence kernel for Trainium2.

u_t = TAU * u_{t-1} + x_t along the last (time) axis of x[32, 1024, 2048] f32.

Strategy: data-parallel across 8 NeuronCores (4 batches each). The problem is
memory-bound, so HBM traffic is halved by moving data as 16-bit floats (the
2e-2 tolerance dwarfs the quantization error). The recurrence is computed on
the Tensor engine as a *banded matmul*: since TAU^129 < 2e-6, u_t is (to
float precision) a windowed sum u_t = sum_{s=t-255..t} TAU^(t-s) x_s. In a
host-transposed layout xt[time, rows], each 128-step output block i is

    u[i*128+m, r] = sum_{k} A[k, m] * xt[(i-1)*128+k, r]   (cross-block band)
                  + sum_{k} B[k, m] * xt[i*128+k, r]       (triangular band)

with A[k, m] = TAU^(m+128-k), B[k, m] = TAU^(m-k) for k<=m else 0 — two
accumulating 128x128-stationary matmuls per PSUM chunk (block 0 skips A).

Engine assignment: Sync issues input DMAs, Scalar issues output DMAs (two
HWDGE rings, so input prefetch never head-of-line blocks behind output
drain), Tensor does the matmuls, and the PSUM f32 -> SBUF 16-bit downcast
copies are split between Vector and Scalar (each ~46 us; a single engine
at ~92 us would sit on the critical path).

The walrus build in this container allows at most ONE embedded sync-wait
per engine instruction (two on EventSemaphore); Tile's wait assignment can
attach several. _split_excess_waits() hoists the extras onto standalone
EventSemaphore instructions inserted immediately before, on the same
engine — conservative but correct, since every awaited semaphore's
producer precedes the waiter in the scheduled program order.
"""

import numpy as np
import ml_dtypes

import concourse.bass as bass
import concourse.mybir as mybir
from concourse.bass_utils import run_bass_kernel_spmd
from concourse.tile import TileContext

TAU = 0.9
B, F, T = 32, 1024, 2048
N_CORES = 8
B_PER_CORE = B // N_CORES          # 4
ROWS = B_PER_CORE * F              # 4096 independent recurrences per core
P = 128
N_BLK = T // P                     # 16 time-blocks (slabs) per core
CHUNK = 512                        # PSUM bank width (f32)
N_CHUNK = ROWS // CHUNK            # 8

NP_DT = ml_dtypes.bfloat16
MYBIR_DT = mybir.dt.bfloat16

_nc_cache = None
_coef_cache = None
last_results = None  # BassKernelResults from the most recent run (for test.py)


def _split_excess_waits(nc: bass.Bass) -> None:
    for fn in nc.m.functions:
        for blk in fn.blocks:
            out = []
            changed = False
            for inst in blk.instructions:
                si = inst.sync_info
                waits = list(si.on_wait) if si is not None else []
                cap = 2 if inst.opcode == "EventSemaphore" else 1
                if len(waits) <= cap:
                    out.append(inst)
                    continue
                changed = True
                # On DMAs keep a queue-ordering (DMAHW*) wait embedded so
                # queue-level throttling stays at the queue; otherwise keep
                # the last wait.
                keep_idx = len(waits) - 1
                if inst.opcode == "DMACopy":
                    for k, w in enumerate(waits):
                        if (w.ant_name or "").startswith("DMA"):
                            keep_idx = k
                            break
                rest = [w for j, w in enumerate(waits) if j != keep_idx]
                for j in range(0, len(rest), 2):
                    out.append(
                        mybir.InstEventSemaphore(
                            name=f"{inst.name}-xw{j}",
                            opcode="EventSemaphore",
                            engine=inst.engine,
                            debug=inst.debug,
                            sync_info=mybir.SyncInfo(
                                on_wait=rest[j : j + 2], on_update=[]
                            ),
                        )
                    )
                inst.sync_info = mybir.SyncInfo(
                    on_wait=[waits[keep_idx]], on_update=list(si.on_update)
                )
                out.append(inst)
            if changed:
                blk.instructions = out


def _dedup_ldweights(nc: bass.Bass) -> None:
    """Drop PE weight reloads that reload the already-loaded stationary.

    tile_legalize splits every matmul into InstLdweights + a
    non-self-loading InstMatmult. Matmult does not clobber the PE weight
    array, so consecutive Ldweights with an identical weights AP are
    redundant — all but the first can go (saving ~100 ns of PE time each,
    ~21 us total here). A redundant Ldweights that carries semaphore
    waits/updates is replaced by an EventSemaphore on the same engine so
    the synchronization is preserved; any other PE instruction resets the
    tracked signature (conservative).
    """
    for fn in nc.m.functions:
        for blk in fn.blocks:
            out = []
            changed = False
            last_sig = None
            for inst in blk.instructions:
                if inst.opcode == "Matmult":
                    out.append(inst)
                    continue
                if inst.opcode != "Ldweights":
                    if inst.engine == mybir.EngineType.PE and inst.opcode not in (
                        "EventSemaphore",
                    ):
                        last_sig = None
                    out.append(inst)
                    continue
                a = inst.ins[0]
                sig = (a.memref, a.offset, str(a.ap), str(a.dtype))
                if sig != last_sig:
                    last_sig = sig
                    out.append(inst)
                    continue
                changed = True
                si = inst.sync_info
                waits = list(si.on_wait) if si is not None else []
                upds = list(si.on_update) if si is not None else []
                if waits or upds:
                    for j in range(0, max(len(waits), 1), 2):
                        out.append(
                            mybir.InstEventSemaphore(
                                name=f"{inst.name}-lw{j}",
                                opcode="EventSemaphore",
                                engine=inst.engine,
                                debug=inst.debug,
                                sync_info=mybir.SyncInfo(
                                    on_wait=waits[j : j + 2],
                                    on_update=upds if j == 0 else [],
                                ),
                            )
                        )
            if changed:
                blk.instructions = out


def _coef() -> np.ndarray:
    # [P, 2P] = [A | B] packed side by side (one SBUF tile, one DMA):
    #   A[k, m] = TAU^(m+128-k)                (cross-block band)
    #   B[k, m] = TAU^(m-k) for k <= m else 0  (triangular band)
    k = np.arange(2 * P)[:, None]
    m = np.arange(P)[None, :]
    e = m + P - k
    c = np.where(e >= 0, TAU ** np.maximum(e, 0).astype(np.float64), 0.0)
    return np.ascontiguousarray(
        np.hstack([c[:P], c[P:]]).astype(NP_DT)
    )


def _build() -> bass.Bass:
    nc = bass.Bass()
    xt = nc.dram_tensor("xt", [T, ROWS], MYBIR_DT, kind="ExternalInput")
    coef = nc.dram_tensor("coef", [P, 2 * P], MYBIR_DT, kind="ExternalInput")
    yt = nc.dram_tensor("yt", [T, ROWS], MYBIR_DT, kind="ExternalOutput")

    x_r = xt.rearrange("(i p) r -> i p r", p=P)   # 16 slabs [128, ROWS]
    y_r = yt.rearrange("(i p) r -> i p r", p=P)   # 16 blocks [128, ROWS]

    with TileContext(nc) as tc:
        with (
            tc.tile_pool(name="const", bufs=1) as cpool,
            tc.tile_pool(name="in", bufs=8) as ipool,
            tc.tile_pool(name="out", bufs=4) as opool,
            tc.tile_pool(name="psum", bufs=8, space="PSUM") as ppool,
        ):
            cf = cpool.tile([P, 2 * P], MYBIR_DT)
            nc.sync.dma_start(out=cf[:], in_=coef[:])
            cA = cf[:, 0:P]
            cB = cf[:, P : 2 * P]

            LAST = N_BLK - 1
            slabs = []
            for i in range(N_BLK):
                s = ipool.tile([P, ROWS], MYBIR_DT)
                if i == LAST:
                    # Final block: half-granular input and quarter-granular
                    # output so its writes are ready as the read stream ends
                    # (shortens the exposed tail chain).
                    h = ROWS // 2
                    nc.sync.dma_start(out=s[:, 0:h], in_=x_r[i][:, 0:h])
                    nc.sync.dma_start(out=s[:, h:ROWS], in_=x_r[i][:, h:ROWS])
                else:
                    nc.sync.dma_start(out=s[:], in_=x_r[i])
                slabs.append(s)

                utile = opool.tile([P, ROWS], MYBIR_DT)
                # All-A then all-B so the redundant-LDWEIGHTS dedup pass can
                # collapse each group to one weight load; the 8 chunks exactly
                # fill the 8 PSUM banks. Chunk direction alternates per block
                # so block i+1's A-matmuls only become ready (PSUM bank freed)
                # after block i's B-phase — keeping same-weight runs
                # contiguous in the scheduled PE order.
                order = list(range(N_CHUNK))
                if i % 2:
                    order.reverse()
                pts = {}
                for c in order:
                    pt = ppool.tile([P, CHUNK], mybir.dt.float32)
                    pts[c] = pt
                    sl = slice(c * CHUNK, (c + 1) * CHUNK)
                    if i > 0:
                        nc.tensor.matmul(
                            pt[:], lhsT=cA[:], rhs=slabs[i - 1][:, sl],
                            start=True, stop=False,
                        )
                copied = set()
                for c in order:
                    sl = slice(c * CHUNK, (c + 1) * CHUNK)
                    nc.tensor.matmul(
                        pts[c][:], lhsT=cB[:], rhs=slabs[i][:, sl],
                        start=(i == 0), stop=True,
                    )
                    if c % 2 == 0:
                        nc.vector.tensor_copy(utile[:, sl], pts[c][:])
                    else:
                        nc.scalar.copy(utile[:, sl], pts[c][:])
                    copied.add(c)
                    if i == LAST and (c ^ 1) in copied:
                        # final block streams output per chunk-pair so its
                        # writes are ready as the read stream ends
                        base = min(c, c ^ 1)
                        qs = slice(base * CHUNK, (base + 2) * CHUNK)
                        nc.scalar.dma_start(out=y_r[i][:, qs], in_=utile[:, qs])
                if i != LAST:
                    nc.scalar.dma_start(out=y_r[i], in_=utile[:])
                if i >= 1:
                    slabs[i - 1] = None

    _dedup_ldweights(nc)
    _split_excess_waits(nc)
    return nc


def kernel(x: np.ndarray, **_unused) -> np.ndarray:
    global _nc_cache, _coef_cache, last_results
    if _nc_cache is None:
        _nc_cache = _build()
        _coef_cache = _coef()
    nc = _nc_cache

    x = np.asarray(x)
    assert x.shape == (B, F, T), x.shape
    x16 = np.ascontiguousarray(x.reshape(N_CORES, ROWS, T), dtype=NP_DT)
    in_maps = [
        {"xt": np.ascontiguousarray(x16[c].T), "coef": _coef_cache}
        for c in range(N_CORES)
    ]
    last_results = run_bass_kernel_spmd(
        nc, in_maps, core_ids=list(range(N_CORES))
    )
    out = np.concatenate(
        [
            r["yt"].T.astype(np.float32).reshape(B_PER_CORE, F, T)
            for r in last_results.results
        ],
        axis=0,
    )
    return out
